# revision 76
# baseline (speedup 1.0000x reference)
"""Trainium2 Bass kernel for multi-head attention (b=4, n=2048, d=512, h=8, dk=dv=64).

Sharding: 8 cores = 4 batches x 2 query-halves. Each core computes K/V for its
full batch sequence (2048) and attention outputs for its 1024 query rows.
No collectives needed; host stacks the per-core [1024, 512] outputs.

Per-core dataflow (all matmul operands fp16 except P/V in bf16; fp16 keeps
f32r-class precision - 10 mantissa bits - while its 2-byte LDWEIGHTS streams
at full PE rate, where 4-byte f32r stationary loads stall the PE ~130ns per
matmul):
  x^T [512, 2048] staged in SBUF as fp16.
  Head-PAIR packed projections: Q^T/K^T per pair p (heads 2p, 2p+1):
    lhsT = w[:, ch, p*128:(p+1)*128] -> out [128 rows = headA 64 dims | headB
    64 dims, n].  Rel-bias is folded into Q^T via a per-partition scalar add.
  K^T stored per head in [128, 2048] fp16 tiles with the OTHER head's
    partition half zeroed, so every ST matmul is a uniform 128x128 tile
    config (mixed 64-row/128-row configs cost ~200ns per switch):
    lhsT = kt_h[h][:, jc*128:+128], rhs = qt [128, 512i] -> S^T [128j, 512i].
    The zero half multiplies the other head's qt rows to zero contribution.
  V = x Wv (+ ones col) [per j-chunk: 128j, 8h*65] in bf16.
  P^T = exp(S^T) -> bf16 (no max-subtraction: logits < ~50; bf16 range ok).
    One ACT instruction per 2 j-chunks ([128, 1024], from a 2-bank PSUM st
    tile) amortizes the ~260ns fixed Activation-engine cost.
  PV (bf16) accumulated over 16 j-chunks into [65, 512] PSUM per (head,
    i-half); row 64 = denominator (ones column of V_aug).  ST/exp/PV run as
    ONE continuous stream over all 256 (head, i-half, batch) units with PV
    lagging ST by 13 batches (deep pt buffering), so the exp semaphore is
    always satisfied when PV issues and the pipeline never restarts at
    (head, i-half) boundaries.  NOTE: pool tiles allocated in loops must use
    a CONSTANT name= — unique per-iteration names break buffer rotation.
  Normalization: reciprocal_approx_fast + gpsimd partition broadcast + DVE
    multiply -> outt (fp16, one tile per head pair so the output projection
    only waits on the pairs it reads).
  y = outt^T @ Wo + bo, accumulated over head pairs; the first i-half is
    projected during head 7's second half to shorten the tail.

Schedule: head h's Q/K projections are emitted during head h-2 (pair-ahead);
V projections and pair-0's K groups 2-3 are interleaved into head 0's ST
stream so the PE never waits on late x/wv DMA chunks.  Input DMAs are
priority-ordered (wq, x[own queries], wk, wv, x[rest], wo) across the
sync/gpsimd/scalar queues.

PSUM budget (8 banks): st pool 2 bufs x [128, 2jc*512] f32 (2 banks each)
+ qk pool 2 bufs x [128, 512] (1 bank each) + pv 2 bufs x [65, 512] (1 bank).
"""
import numpy as np

B, N, MODEL = 4, 2048, 512
H, DK = 8, 64
SCALE = DK ** -0.5
NP = H // 2         # head pairs
NI = 1024           # query rows per core
NCH = MODEL // 128  # model-dim chunks
NJC = N // 128      # key/value chunks
JB = 2              # j-chunks per ST/exp batch
NB = NJC // JB      # batches per (head, i-half)

_COMPILED = None


def _build():
    import concourse.bass as bass
    from concourse import bacc
    import concourse.mybir as mybir
    import concourse.tile as tile

    F32 = mybir.dt.float32
    F32R = mybir.dt.float32r
    BF16 = mybir.dt.bfloat16
    FP16 = mybir.dt.float16
    EXP = mybir.ActivationFunctionType.Exp

    # Inputs are pre-swizzled on the host to per-partition-contiguous layouts
    # (partition dim first) so every DMA moves multi-KB contiguous runs at
    # full HBM bandwidth instead of descriptor-dominated 1KB strides.
    nc = bacc.Bacc("TRN2", target_bir_lowering=False, debug=False, num_devices=8)
    xt_in = nc.dram_tensor("xt", [128, 16, 512], FP16, kind="ExternalInput")
    wq_in = nc.dram_tensor("wq", [128, NCH, MODEL], FP16, kind="ExternalInput")
    wk_in = nc.dram_tensor("wk", [128, NCH, MODEL], FP16, kind="ExternalInput")
    wv_in = nc.dram_tensor("wv", [128, NCH, MODEL], FP16, kind="ExternalInput")
    relb_in = nc.dram_tensor("relb", [128, NP], F32, kind="ExternalInput")
    wo_in = nc.dram_tensor("wo", [128, NP, MODEL], FP16, kind="ExternalInput")
    bo_in = nc.dram_tensor("bo", [1, MODEL], F32, kind="ExternalInput")
    onesb_in = nc.dram_tensor("onesb", [128, NJC * H], BF16, kind="ExternalInput")
    y_out = nc.dram_tensor("y", [NI, MODEL], F32, kind="ExternalOutput")

    with tile.TileContext(nc) as tc:
        with (
            tc.tile_pool(name="w", bufs=1) as wp,
            tc.tile_pool(name="acts", bufs=1) as ap,
            tc.tile_pool(name="st", bufs=2, space="PSUM") as stp,
            tc.tile_pool(name="qk", bufs=2, space="PSUM") as qkp,
            tc.tile_pool(name="pv", bufs=2, space="PSUM") as pvp,
        ):
            # ---------- persistent tiles ----------
            wq_c = [wp.tile([128, MODEL], FP16, name=f"wq{i}", tag=f"wq{i}")
                    for i in range(NCH)]
            wk = wp.tile([128, NCH, MODEL], FP16, tag="wk")
            wv = wp.tile([128, NCH, MODEL], FP16, tag="wv")
            wo = wp.tile([128, NP, MODEL], FP16, tag="wo")
            relb = wp.tile([128, NP], F32, tag="relb")
            bo = wp.tile([1, MODEL], F32, tag="bo")
            bo_b = wp.tile([128, MODEL], F32, tag="bo_b")
            onesb_t = wp.tile([128, NJC * H], BF16, tag="onesb")
            warm = wp.tile([128, 512], FP16, tag="warm")

            xt0 = ap.tile([128, NCH, 512], FP16, tag="xt0")
            xt1 = ap.tile([128, NCH, 512], FP16, tag="xt1")
            xt2 = ap.tile([128, NCH, 512], FP16, tag="xt2")
            xt3 = ap.tile([128, NCH, 512], FP16, tag="xt3")
            xts = [xt0, xt1, xt2, xt3]
            vv_t = [ap.tile([128, 4, H * 65], BF16, name=f"vv{i}",
                            tag=f"vv{i}") for i in range(4)]
            qt_t = [[ap.tile([128, 512], FP16, name=f"qt{i}{g}",
                             tag=f"qt{i}{g}") for g in range(2)]
                    for i in range(2)]
            kt_h = [ap.tile([128, NJC * 128], FP16, name=f"kt_h{i}",
                            tag=f"kt{i}") for i in range(4)]
            outt_p = [ap.tile([128, NI], FP16, name=f"outt{i}", tag=f"outt{i}")
                      for i in range(NP)]

            def vvt(jc):
                return vv_t[jc // 4][:, jc % 4]

            def r3(d):
                return d[:].rearrange("(c p) n -> p c n", p=128)

            def xtv(ch, start, size):
                t = xts[start // 512]
                off = start % 512
                assert off + size <= 512
                return t[:, ch, off:off + size]

            # ---------- input staging: ONE sync queue, strict FIFO priority
            # (a queued transfer gets the full ~300GB/s; in-queue order IS
            # the priority).  First ST needs wq + xt[g0,g1] + wk. ----------
            nc.sync.dma_start(out=wq_c[0][:], in_=wq_in[:, 0])
            nc.sync.dma_start(out=wq_c[1][:], in_=wq_in[:, 1])
            nc.sync.dma_start(out=wq_c[2][:], in_=wq_in[:, 2])
            nc.sync.dma_start(out=wq_c[3][:], in_=wq_in[:, 3])
            nc.sync.dma_start(out=relb[:], in_=relb_in[:])
            nc.sync.dma_start(out=xts[0][:], in_=xt_in[:, 0:4, :])
            nc.sync.dma_start(out=wk[:], in_=wk_in[:])
            nc.sync.dma_start(out=xts[1][:], in_=xt_in[:, 4:8, :])
            nc.sync.dma_start(out=wv[:], in_=wv_in[:])
            nc.sync.dma_start(out=xts[2][:], in_=xt_in[:, 8:12, :])
            nc.sync.dma_start(out=xts[3][:], in_=xt_in[:, 12:16, :])
            nc.sync.dma_start(out=onesb_t[:], in_=onesb_in[:])
            nc.sync.dma_start(out=wo[:], in_=wo_in[:])
            nc.sync.dma_start(out=bo[:], in_=bo_in[:])
            # ones columns of V_aug: contiguous DMA to scratch, strided DVE copy
            for vh in range(4):
                nc.gpsimd.tensor_copy(
                    vv_t[vh][:]
                    .rearrange("p j (h e) -> p (j h) e", e=65)[:, :, 64:65],
                    onesb_t[:, vh * 4 * H:(vh + 1) * 4 * H]
                    .rearrange("p (n o) -> p n o", o=1))
            nc.gpsimd.partition_broadcast(bo_b[:], bo[:])
            nc.vector.memset(warm[:], 1.0)

            with (
                tc.tile_pool(name="pt", bufs=14) as ptp,
                tc.tile_pool(name="norm", bufs=2) as np_,
                tc.tile_pool(name="ysb", bufs=2) as yp_sb,
            ):
                # ---- head-pair packed Q^T / K^T projections (f32r) ----
                def emit_qk(p, kgs=(0, 1, 2, 3), qgs=(0, 1), sc_copy=False):
                    qt = qt_t[p % 2]
                    ktA = kt_h[(2 * p) % 4]
                    ktB = kt_h[(2 * p + 1) % 4]
                    cols = slice(p * 128, (p + 1) * 128)
                    for g in qgs:
                        q_ps = qkp.tile([128, 512], F32, tag="qk")
                        for ch in range(NCH):
                            nc.tensor.matmul(
                                q_ps[:], wq_c[ch][:, cols],
                                xtv(ch, g * 512, 512),
                                start=(ch == 0), stop=(ch == NCH - 1))
                        nc.vector.tensor_scalar_add(
                            qt[g][:], q_ps[:], relb[:, p:p + 1])
                    for g in kgs:
                        k_ps = qkp.tile([128, 512], F32, tag="qk")
                        for ch in range(NCH):
                            nc.tensor.matmul(
                                k_ps[:], wk[:, ch, cols], xtv(ch, g * 512, 512),
                                start=(ch == 0), stop=(ch == NCH - 1))
                        ceng = nc.scalar if sc_copy else nc.vector
                        if sc_copy:
                            nc.scalar.copy(
                                ktA[0:64, g * 512:(g + 1) * 512], k_ps[0:64, :])
                            nc.scalar.copy(
                                ktB[64:128, g * 512:(g + 1) * 512],
                                k_ps[64:128, :])
                        else:
                            nc.vector.tensor_copy(
                                ktA[0:64, g * 512:(g + 1) * 512], k_ps[0:64, :])
                            nc.vector.tensor_copy(
                                ktB[64:128, g * 512:(g + 1) * 512],
                                k_ps[64:128, :])

                def emit_v(bi):
                    for jc in range(bi * JB * 2, (bi + 1) * JB * 2):
                        v_ps = qkp.tile([128, 512], F32, tag="qk")
                        for ch in range(NCH):
                            nc.tensor.matmul(
                                v_ps[:], xtv(ch, jc * 128, 128), wv[:, ch],
                                start=(ch == 0), stop=(ch == NCH - 1))
                        nc.vector.tensor_copy(
                            vvt(jc).rearrange("p (h e) -> p h e", e=65)[:, :, 0:64],
                            v_ps[:].rearrange("p (h e) -> p h e", e=64))

                def emit_yproj(ibs):
                    for ib in ibs:
                        y_ps = qkp.tile([128, 512], F32, tag="qk")
                        for hp2 in range(NP):
                            nc.tensor.matmul(
                                y_ps[:],
                                outt_p[hp2][:, ib * 128:(ib + 1) * 128],
                                wo[:, hp2], start=(hp2 == 0),
                                stop=(hp2 == NP - 1))
                        y_sb = yp_sb.tile([128, MODEL], F32, tag="ysb")
                        nc.vector.tensor_tensor(out=y_sb[:], in0=y_ps[:],
                                                in1=bo_b[:],
                                                op=mybir.AluOpType.add)
                        nc.sync.dma_start(
                            out=y_out[ib * 128:(ib + 1) * 128, :], in_=y_sb[:])

                # zero the pad halves of the per-head K tiles once
                for i in range(4):
                    half = slice(64, 128) if i % 2 == 0 else slice(0, 64)
                    nc.gpsimd.memset(kt_h[i][half, :], 0.0)

                # PE warm-up during the DMA wait: ~5 cold matmuls (~3.2us)
                # keep the HAM activity window busy so the clock is already
                # 2.4GHz when the first real matmul's inputs land; one
                # accumulation group so they pipeline; results never read.
                wu = qkp.tile([128, 512], F32, name="warmup", tag="qk")
                for i in range(5):
                    nc.tensor.matmul(wu[:], warm[:, 0:128], warm[:],
                                     start=(i == 0), stop=(i == 4))

                # slot 0 needs only q(g0)+k(g0): q(g1)/k(g1..3) (which would
                # wait on late xt groups and head-of-line-block the first ST
                # in the FIFO tensor queue) move into the stream
                emit_qk(0, qgs=(0,), kgs=(0,))

                LAG = 13
                TOT = H * 2 * NB
                pv_next = 0
                pv_tiles = {}
                pts = {}
                for g in range(TOT + LAG):
                    if g < TOT:
                        u, bi = divmod(g, NB)
                        h, ih = divmod(u, 2)
                        hp = h // 2
                        qt = qt_t[hp % 2]
                        kt = kt_h[h % 4]
                        if bi == 0:
                            pv_tiles[u] = pvp.tile([65, 512], F32, name="pv",
                                                   tag="pv")
                        st = stp.tile([128, JB * 512], F32, tag="st")
                        for k in range(JB):
                            jc = bi * JB + k
                            nc.tensor.matmul(
                                st[:, k * 512:(k + 1) * 512],
                                kt[:, jc * 128:(jc + 1) * 128],
                                qt[ih][:], start=True, stop=True)
                        pt = ptp.tile([128, JB * 512], BF16, name="pt",
                                      tag="pt")
                        pts[g] = pt
                        nc.scalar.activation(pt[:], st[:], EXP, scale=1.0)
                        if h == 0 and ih == 0 and bi == 0:
                            emit_qk(0, qgs=(), kgs=(1,), sc_copy=True)
                        if h == 0 and ih == 0 and bi < 4:
                            emit_v(bi)
                        if h == 0 and ih == 0 and bi == 1:
                            emit_qk(0, qgs=(1,), kgs=(2, 3), sc_copy=True)
                        if (ih == 0 and h % 2 == 0 and h + 2 < H
                                and bi == (2 if h == 0 else 0)):
                            emit_qk(hp + 1, sc_copy=(h == 0))
                    # PV lags the exp stream by LAG groups in steady state
                    # (sem always satisfied at issue); the lag TAPERS to 3
                    # over the last slots so the post-exp drain is short.
                    lag_eff = (LAG if g < TOT - 5
                               else max(3, LAG - 2 * (g - (TOT - 5))))
                    while pv_next < TOT and pv_next <= g - lag_eff:
                        gp = pv_next
                        pv_next += 1
                        up, bip = divmod(gp, NB)
                        hh, ihp = divmod(up, 2)
                        ptb = pts.pop(gp)
                        for k in range(JB):
                            jc = bip * JB + k
                            nc.tensor.matmul(
                                pv_tiles[up][:],
                                vvt(jc)[:, hh * 65:(hh + 1) * 65],
                                ptb[:, k * 512:(k + 1) * 512],
                                start=(jc == 0), stop=(jc == NJC - 1))
                        if bip == NB - 1:
                            pv_d = pv_tiles.pop(up)
                            hp2, hr2 = hh // 2, (hh % 2) * 64
                            isl2 = slice(ihp * 512, (ihp + 1) * 512)
                            den = np_.tile([1, 512], F32, tag="den")
                            nc.vector.tensor_copy(den[:], pv_d[64:65, :])
                            rrow = np_.tile([1, 512], F32, tag="rrow")
                            nc.vector.reciprocal_approx_fast(rrow[:], den[:])
                            rb = np_.tile([64, 512], F32, tag="rb")
                            nc.gpsimd.partition_broadcast(rb[:], rrow[:])
                            nc.vector.tensor_tensor(
                                out=outt_p[hp2][hr2:hr2 + 64, isl2],
                                in0=pv_d[0:64, :], in1=rb[:],
                                op=mybir.AluOpType.mult)
                            if hh == H - 1 and ihp == 0:
                                emit_yproj(range(4))

                # keep-alive matmuls: the serialized final norm chain would
                # otherwise idle the PE past the HAM window, making the last
                # y projections run at 1.2GHz
                wu2 = qkp.tile([128, 512], F32, name="warmup2", tag="qk")
                for i in range(10):
                    nc.tensor.matmul(wu2[:], warm[:, 0:128], warm[:],
                                     start=(i == 0), stop=(i == 9))
                emit_yproj(range(4, NI // 128))

    nc.compile()
    return nc


def _get_compiled():
    global _COMPILED
    if _COMPILED is None:
        _COMPILED = _build()
    return _COMPILED


def kernel(x, Wq, Wk, Wv, Wo, bo, rel_content_bias, _trace=False):
    from concourse.bass_utils import run_bass_kernel_spmd
    import ml_dtypes

    nc = _get_compiled()

    x = np.asarray(x, dtype=np.float32)
    Wq = np.asarray(Wq, dtype=np.float32)
    Wk = np.asarray(Wk, dtype=np.float32)
    Wv = np.asarray(Wv, dtype=np.float32)
    Wo = np.asarray(Wo, dtype=np.float32)
    bo = np.asarray(bo, dtype=np.float32)
    bias = np.asarray(rel_content_bias, dtype=np.float32).reshape(H, DK)

    Wq_s = (Wq * SCALE).astype(np.float32)
    # relb column p = [bias of head 2p (64) | bias of head 2p+1 (64)]
    relb = bias.reshape(NP, 2 * DK).T.astype(np.float32)  # [128, NP]
    onesb = np.ones((128, NJC * H), ml_dtypes.bfloat16)

    # Host-side swizzle to per-partition-contiguous DMA layouts:
    # W[ch*128+p, n] -> [p, ch, n]
    def w3(w):
        return np.ascontiguousarray(
            w.astype(np.float16).reshape(NCH, 128, MODEL).transpose(1, 0, 2))

    shared = {"wq": w3(Wq_s), "wk": w3(Wk), "wv": w3(Wv), "relb": relb,
              "wo": w3(Wo), "bo": bo[None, :], "onesb": onesb}

    in_maps = []
    for c in range(8):
        b, half = c // 2, c % 2
        xt = x[b].T.astype(np.float16)                         # [512, 2048]
        if half:
            xt = np.roll(xt, -NI, axis=1)
        # x^T[ch*128+p, g*512+n] -> [p, g*4+ch, n]
        xt = np.ascontiguousarray(
            xt.reshape(NCH, 128, 4, 512).transpose(1, 2, 0, 3)
            .reshape(128, 16, 512))
        in_maps.append({"xt": xt, **shared})

    res = run_bass_kernel_spmd(nc, in_maps, core_ids=list(range(8)),
                               trace=_trace)
    out = np.empty((B, N, MODEL), np.float32)
    for c in range(8):
        b, half = c // 2, c % 2
        out[b, half * NI:(half + 1) * NI, :] = res.results[c]["y"]
    if _trace:
        return out, res
    return out



# revision 79
# speedup vs baseline: 1.1951x; 1.1951x over previous
"""Trainium2 Bass kernel for multi-head attention (b=4, n=2048, d=512, h=8, dk=dv=64).

Sharding: 8 cores = 4 batches x 2 query-halves. Each core computes K/V for its
full batch sequence (2048) and attention outputs for its 1024 query rows.
No collectives needed; host stacks the per-core [1024, 512] outputs.

Per-core dataflow (all matmul operands fp16 except P/V in bf16; fp16 keeps
f32r-class precision - 10 mantissa bits - while its 2-byte LDWEIGHTS streams
at full PE rate, where 4-byte f32r stationary loads stall the PE ~130ns per
matmul):
  x^T [512, 2048] staged in SBUF as fp16.
  Head-PAIR packed projections: Q^T/K^T per pair p (heads 2p, 2p+1):
    lhsT = w[:, ch, p*128:(p+1)*128] -> out [128 rows = headA 64 dims | headB
    64 dims, n].  Rel-bias is folded into Q^T via a per-partition scalar add.
  K^T stored per head in [128, 2048] fp16 tiles with the OTHER head's
    partition half zeroed, so every ST matmul is a uniform 128x128 tile
    config (mixed 64-row/128-row configs cost ~200ns per switch):
    lhsT = kt_h[h][:, jc*128:+128], rhs = qt [128, 512i] -> S^T [128j, 512i].
    The zero half multiplies the other head's qt rows to zero contribution.
  V = x Wv (+ ones col) [per j-chunk: 128j, 8h*65] in bf16.
  P^T = exp(S^T) -> bf16 (no max-subtraction: logits < ~50; bf16 range ok).
    One ACT instruction per 2 j-chunks ([128, 1024], from a 2-bank PSUM st
    tile) amortizes the ~260ns fixed Activation-engine cost.
  PV (bf16) accumulated over 16 j-chunks into [65, 512] PSUM per (head,
    i-half); row 64 = denominator (ones column of V_aug).  ST/exp/PV run as
    ONE continuous stream over all 256 (head, i-half, batch) units with PV
    lagging ST by 13 batches (deep pt buffering), so the exp semaphore is
    always satisfied when PV issues and the pipeline never restarts at
    (head, i-half) boundaries.  NOTE: pool tiles allocated in loops must use
    a CONSTANT name= — unique per-iteration names break buffer rotation.
  Normalization: reciprocal_approx_fast + gpsimd partition broadcast + DVE
    multiply -> outt (fp16, one tile per head pair so the output projection
    only waits on the pairs it reads).
  y = outt^T @ Wo + bo, accumulated over head pairs; the first i-half is
    projected during head 7's second half to shorten the tail.

Schedule: head h's Q/K projections are emitted during head h-2 (pair-ahead);
V projections and pair-0's K groups 2-3 are interleaved into head 0's ST
stream so the PE never waits on late x/wv DMA chunks.  Input DMAs are
priority-ordered (wq, x[own queries], wk, wv, x[rest], wo) across the
sync/gpsimd/scalar queues.

PSUM budget (8 banks): st pool 2 bufs x [128, 2jc*512] f32 (2 banks each)
+ qk pool 2 bufs x [128, 512] (1 bank each) + pv 2 bufs x [65, 512] (1 bank).
"""
import numpy as np

B, N, MODEL = 4, 2048, 512
H, DK = 8, 64
SCALE = DK ** -0.5
NP = H // 2         # head pairs
NI = 1024           # query rows per core
NCH = MODEL // 128  # model-dim chunks
NJC = N // 128      # key/value chunks
JB = 2              # j-chunks per ST/exp batch
NB = NJC // JB      # batches per (head, i-half)

_COMPILED = None


def _build():
    import concourse.bass as bass
    from concourse import bacc
    import concourse.mybir as mybir
    import concourse.tile as tile

    F32 = mybir.dt.float32
    F32R = mybir.dt.float32r
    BF16 = mybir.dt.bfloat16
    FP16 = mybir.dt.float16
    EXP = mybir.ActivationFunctionType.Exp

    # Inputs are pre-swizzled on the host to per-partition-contiguous layouts
    # (partition dim first) so every DMA moves multi-KB contiguous runs at
    # full HBM bandwidth instead of descriptor-dominated 1KB strides.
    nc = bacc.Bacc("TRN2", target_bir_lowering=False, debug=False, num_devices=8)
    xt_in = nc.dram_tensor("xt", [128, 16, 512], FP16, kind="ExternalInput")
    wq_in = nc.dram_tensor("wq", [128, NCH, MODEL], FP16, kind="ExternalInput")
    wk_in = nc.dram_tensor("wk", [128, NCH, MODEL], FP16, kind="ExternalInput")
    wv_in = nc.dram_tensor("wv", [128, NCH, MODEL], FP16, kind="ExternalInput")
    relb_in = nc.dram_tensor("relb", [128, NP], F32, kind="ExternalInput")
    wo_in = nc.dram_tensor("wo", [128, NP, MODEL], FP16, kind="ExternalInput")
    bo_in = nc.dram_tensor("bo", [1, MODEL], F32, kind="ExternalInput")
    onesb_in = nc.dram_tensor("onesb", [128, NJC * H], BF16, kind="ExternalInput")
    y_out = nc.dram_tensor("y", [NI, MODEL], F32, kind="ExternalOutput")

    with tile.TileContext(nc) as tc:
        with (
            tc.tile_pool(name="w", bufs=1) as wp,
            tc.tile_pool(name="acts", bufs=1) as ap,
            tc.tile_pool(name="st", bufs=2, space="PSUM") as stp,
            tc.tile_pool(name="qk", bufs=2, space="PSUM") as qkp,
            tc.tile_pool(name="pv", bufs=2, space="PSUM") as pvp,
        ):
            # ---------- persistent tiles ----------
            wq_c = [wp.tile([128, MODEL], FP16, name=f"wq{i}", tag=f"wq{i}")
                    for i in range(NCH)]
            wk = wp.tile([128, NCH, MODEL], FP16, tag="wk")
            wv = wp.tile([128, NCH, MODEL], FP16, tag="wv")
            wo = wp.tile([128, NP, MODEL], FP16, tag="wo")
            relb = wp.tile([128, NP], F32, tag="relb")
            bo = wp.tile([1, MODEL], F32, tag="bo")
            bo_b = wp.tile([128, MODEL], F32, tag="bo_b")
            onesb_t = wp.tile([128, NJC * H], BF16, tag="onesb")
            warm = wp.tile([128, 512], FP16, tag="warm")

            xt0 = ap.tile([128, NCH, 512], FP16, tag="xt0")
            xt1 = ap.tile([128, NCH, 512], FP16, tag="xt1")
            xt2 = ap.tile([128, NCH, 512], FP16, tag="xt2")
            xt3 = ap.tile([128, NCH, 512], FP16, tag="xt3")
            xts = [xt0, xt1, xt2, xt3]
            vv_t = [ap.tile([128, 4, H * 65], BF16, name=f"vv{i}",
                            tag=f"vv{i}") for i in range(4)]
            qt_t = [[ap.tile([128, 512], FP16, name=f"qt{i}{g}",
                             tag=f"qt{i}{g}") for g in range(2)]
                    for i in range(2)]
            kt_h = [ap.tile([128, NJC * 128], FP16, name=f"kt_h{i}",
                            tag=f"kt{i}") for i in range(4)]
            outt_p = [ap.tile([128, NI], FP16, name=f"outt{i}", tag=f"outt{i}")
                      for i in range(NP)]

            def vvt(jc):
                return vv_t[jc // 4][:, jc % 4]

            def r3(d):
                return d[:].rearrange("(c p) n -> p c n", p=128)

            def xtv(ch, start, size):
                t = xts[start // 512]
                off = start % 512
                assert off + size <= 512
                return t[:, ch, off:off + size]

            # ---------- input staging: ONE sync queue, strict FIFO priority
            # (a queued transfer gets the full ~300GB/s; in-queue order IS
            # the priority).  First ST needs wq + xt[g0,g1] + wk. ----------
            nc.sync.dma_start(out=wq_c[0][:], in_=wq_in[:, 0])
            nc.sync.dma_start(out=wq_c[1][:], in_=wq_in[:, 1])
            nc.sync.dma_start(out=wq_c[2][:], in_=wq_in[:, 2])
            nc.sync.dma_start(out=wq_c[3][:], in_=wq_in[:, 3])
            nc.sync.dma_start(out=relb[:], in_=relb_in[:])
            nc.sync.dma_start(out=xts[0][:], in_=xt_in[:, 0:4, :])
            nc.sync.dma_start(out=wk[:], in_=wk_in[:])
            nc.sync.dma_start(out=xts[1][:], in_=xt_in[:, 4:8, :])
            nc.sync.dma_start(out=wv[:], in_=wv_in[:])
            nc.sync.dma_start(out=xts[2][:], in_=xt_in[:, 8:12, :])
            nc.sync.dma_start(out=xts[3][:], in_=xt_in[:, 12:16, :])
            nc.sync.dma_start(out=onesb_t[:], in_=onesb_in[:])
            nc.sync.dma_start(out=wo[:], in_=wo_in[:])
            nc.sync.dma_start(out=bo[:], in_=bo_in[:])
            # ones columns of V_aug: contiguous DMA to scratch, strided DVE copy
            for vh in range(4):
                nc.gpsimd.tensor_copy(
                    vv_t[vh][:]
                    .rearrange("p j (h e) -> p (j h) e", e=65)[:, :, 64:65],
                    onesb_t[:, vh * 4 * H:(vh + 1) * 4 * H]
                    .rearrange("p (n o) -> p n o", o=1))
            nc.gpsimd.partition_broadcast(bo_b[:], bo[:])
            nc.vector.memset(warm[:], 1.0)

            with (
                tc.tile_pool(name="pt", bufs=14) as ptp,
                tc.tile_pool(name="norm", bufs=2) as np_,
                tc.tile_pool(name="ysb", bufs=2) as yp_sb,
            ):
                # ---- head-pair packed Q^T / K^T projections (f32r) ----
                def emit_qk(p, kgs=(0, 1, 2, 3), qgs=(0, 1), sc_copy=False):
                    qt = qt_t[p % 2]
                    ktA = kt_h[(2 * p) % 4]
                    ktB = kt_h[(2 * p + 1) % 4]
                    cols = slice(p * 128, (p + 1) * 128)
                    for g in qgs:
                        q_ps = qkp.tile([128, 512], F32, tag="qk")
                        for ch in range(NCH):
                            nc.tensor.matmul(
                                q_ps[:], wq_c[ch][:, cols],
                                xtv(ch, g * 512, 512),
                                start=(ch == 0), stop=(ch == NCH - 1))
                        nc.vector.tensor_scalar_add(
                            qt[g][:], q_ps[:], relb[:, p:p + 1])
                    for g in kgs:
                        k_ps = qkp.tile([128, 512], F32, tag="qk")
                        for ch in range(NCH):
                            nc.tensor.matmul(
                                k_ps[:], wk[:, ch, cols], xtv(ch, g * 512, 512),
                                start=(ch == 0), stop=(ch == NCH - 1))
                        ceng = nc.scalar if sc_copy else nc.vector
                        if sc_copy:
                            nc.scalar.copy(
                                ktA[0:64, g * 512:(g + 1) * 512], k_ps[0:64, :])
                            nc.scalar.copy(
                                ktB[64:128, g * 512:(g + 1) * 512],
                                k_ps[64:128, :])
                        else:
                            nc.vector.tensor_copy(
                                ktA[0:64, g * 512:(g + 1) * 512], k_ps[0:64, :])
                            nc.vector.tensor_copy(
                                ktB[64:128, g * 512:(g + 1) * 512],
                                k_ps[64:128, :])

                def emit_v(bi):
                    for jc in range(bi * JB * 2, (bi + 1) * JB * 2):
                        v_ps = qkp.tile([128, 512], F32, tag="qk")
                        for ch in range(NCH):
                            nc.tensor.matmul(
                                v_ps[:], xtv(ch, jc * 128, 128), wv[:, ch],
                                start=(ch == 0), stop=(ch == NCH - 1))
                        nc.vector.tensor_copy(
                            vvt(jc).rearrange("p (h e) -> p h e", e=65)[:, :, 0:64],
                            v_ps[:].rearrange("p (h e) -> p h e", e=64))

                def emit_yproj(ibs):
                    for ib in ibs:
                        y_ps = qkp.tile([128, 512], F32, tag="qk")
                        for hp2 in range(NP):
                            nc.tensor.matmul(
                                y_ps[:],
                                outt_p[hp2][:, ib * 128:(ib + 1) * 128],
                                wo[:, hp2], start=(hp2 == 0),
                                stop=(hp2 == NP - 1))
                        y_sb = yp_sb.tile([128, MODEL], F32, tag="ysb")
                        nc.vector.tensor_tensor(out=y_sb[:], in0=y_ps[:],
                                                in1=bo_b[:],
                                                op=mybir.AluOpType.add)
                        nc.sync.dma_start(
                            out=y_out[ib * 128:(ib + 1) * 128, :], in_=y_sb[:])

                # zero the pad halves of the per-head K tiles once
                for i in range(4):
                    half = slice(64, 128) if i % 2 == 0 else slice(0, 64)
                    nc.gpsimd.memset(kt_h[i][half, :], 0.0)

                # PE warm-up during the DMA wait: ~5 cold matmuls (~3.2us)
                # keep the HAM activity window busy so the clock is already
                # 2.4GHz when the first real matmul's inputs land; one
                # accumulation group so they pipeline; results never read.
                wu = qkp.tile([128, 512], F32, name="warmup", tag="qk")
                for i in range(5):
                    nc.tensor.matmul(wu[:], warm[:, 0:128], warm[:],
                                     start=(i == 0), stop=(i == 4))

                # slot 0 needs only q(g0)+k(g0): q(g1) (which would wait on
                # xt[g1] and head-of-line-block k(g0)) moves into the stream
                emit_qk(0, qgs=(0,), kgs=(0, 1))

                LAG = 13
                TOT = H * 2 * NB
                pv_next = 0
                pv_tiles = {}
                pts = {}
                for g in range(TOT + LAG):
                    if g < TOT:
                        u, bi = divmod(g, NB)
                        h, ih = divmod(u, 2)
                        hp = h // 2
                        qt = qt_t[hp % 2]
                        kt = kt_h[h % 4]
                        if bi == 0:
                            pv_tiles[u] = pvp.tile([65, 512], F32, name="pv",
                                                   tag="pv")
                        st = stp.tile([128, JB * 512], F32, tag="st")
                        for k in range(JB):
                            jc = bi * JB + k
                            nc.tensor.matmul(
                                st[:, k * 512:(k + 1) * 512],
                                kt[:, jc * 128:(jc + 1) * 128],
                                qt[ih][:], start=True, stop=True)
                        pt = ptp.tile([128, JB * 512], BF16, name="pt",
                                      tag="pt")
                        pts[g] = pt
                        nc.scalar.activation(pt[:], st[:], EXP, scale=1.0)
                        if h == 0 and ih == 0 and bi < 4:
                            emit_v(bi)
                        if h == 0 and ih == 0 and bi == 0:
                            # copies on VECTOR: a scalar copy here waits on
                            # k-matmuls gated by late xt DMAs and would block
                            # the exp stream behind it in the Scalar FIFO
                            emit_qk(0, qgs=(1,), kgs=(2, 3), sc_copy=False)
                        if (ih == 0 and h % 2 == 0 and h + 2 < H
                                and bi == (2 if h == 0 else 0)):
                            emit_qk(hp + 1, sc_copy=False)
                    # PV lags the exp stream by LAG groups in steady state
                    # (sem always satisfied at issue); the lag TAPERS to 3
                    # over the last slots so the post-exp drain is short.
                    lag_eff = (LAG if g < TOT - 5
                               else max(3, LAG - 2 * (g - (TOT - 5))))
                    while pv_next < TOT and pv_next <= g - lag_eff:
                        gp = pv_next
                        pv_next += 1
                        up, bip = divmod(gp, NB)
                        hh, ihp = divmod(up, 2)
                        ptb = pts.pop(gp)
                        for k in range(JB):
                            jc = bip * JB + k
                            nc.tensor.matmul(
                                pv_tiles[up][:],
                                vvt(jc)[:, hh * 65:(hh + 1) * 65],
                                ptb[:, k * 512:(k + 1) * 512],
                                start=(jc == 0), stop=(jc == NJC - 1))
                        if bip == NB - 1:
                            pv_d = pv_tiles.pop(up)
                            hp2, hr2 = hh // 2, (hh % 2) * 64
                            isl2 = slice(ihp * 512, (ihp + 1) * 512)
                            den = np_.tile([1, 512], F32, tag="den")
                            nc.vector.tensor_copy(den[:], pv_d[64:65, :])
                            rrow = np_.tile([1, 512], F32, tag="rrow")
                            nc.vector.reciprocal_approx_fast(rrow[:], den[:])
                            rb = np_.tile([64, 512], F32, tag="rb")
                            nc.gpsimd.partition_broadcast(rb[:], rrow[:])
                            nc.vector.tensor_tensor(
                                out=outt_p[hp2][hr2:hr2 + 64, isl2],
                                in0=pv_d[0:64, :], in1=rb[:],
                                op=mybir.AluOpType.mult)
                            if hh == H - 1 and ihp == 0:
                                emit_yproj(range(4))

                # keep-alive matmuls: the serialized final norm chain would
                # otherwise idle the PE past the HAM window, making the last
                # y projections run at 1.2GHz
                wu2 = qkp.tile([128, 512], F32, name="warmup2", tag="qk")
                for i in range(10):
                    nc.tensor.matmul(wu2[:], warm[:, 0:128], warm[:],
                                     start=(i == 0), stop=(i == 9))
                emit_yproj(range(4, NI // 128))

    nc.compile()
    return nc


def _get_compiled():
    global _COMPILED
    if _COMPILED is None:
        _COMPILED = _build()
    return _COMPILED


def kernel(x, Wq, Wk, Wv, Wo, bo, rel_content_bias, _trace=False):
    from concourse.bass_utils import run_bass_kernel_spmd
    import ml_dtypes

    nc = _get_compiled()

    x = np.asarray(x, dtype=np.float32)
    Wq = np.asarray(Wq, dtype=np.float32)
    Wk = np.asarray(Wk, dtype=np.float32)
    Wv = np.asarray(Wv, dtype=np.float32)
    Wo = np.asarray(Wo, dtype=np.float32)
    bo = np.asarray(bo, dtype=np.float32)
    bias = np.asarray(rel_content_bias, dtype=np.float32).reshape(H, DK)

    Wq_s = (Wq * SCALE).astype(np.float32)
    # relb column p = [bias of head 2p (64) | bias of head 2p+1 (64)]
    relb = bias.reshape(NP, 2 * DK).T.astype(np.float32)  # [128, NP]
    onesb = np.ones((128, NJC * H), ml_dtypes.bfloat16)

    # Host-side swizzle to per-partition-contiguous DMA layouts:
    # W[ch*128+p, n] -> [p, ch, n]
    def w3(w):
        return np.ascontiguousarray(
            w.astype(np.float16).reshape(NCH, 128, MODEL).transpose(1, 0, 2))

    shared = {"wq": w3(Wq_s), "wk": w3(Wk), "wv": w3(Wv), "relb": relb,
              "wo": w3(Wo), "bo": bo[None, :], "onesb": onesb}

    in_maps = []
    for c in range(8):
        b, half = c // 2, c % 2
        xt = x[b].T.astype(np.float16)                         # [512, 2048]
        if half:
            xt = np.roll(xt, -NI, axis=1)
        # x^T[ch*128+p, g*512+n] -> [p, g*4+ch, n]
        xt = np.ascontiguousarray(
            xt.reshape(NCH, 128, 4, 512).transpose(1, 2, 0, 3)
            .reshape(128, 16, 512))
        in_maps.append({"xt": xt, **shared})

    res = run_bass_kernel_spmd(nc, in_maps, core_ids=list(range(8)),
                               trace=_trace)
    out = np.empty((B, N, MODEL), np.float32)
    for c in range(8):
        b, half = c // 2, c % 2
        out[b, half * NI:(half + 1) * NI, :] = res.results[c]["y"]
    if _trace:
        return out, res
    return out

